# revision 21
# baseline (speedup 1.0000x reference)
"""MultiHeadSelfAttentionWithLagBias on 8 TRN2 NeuronCores.

Sharding: tensor-parallel over heads — 16 heads / 8 cores = 2 heads per
core. Each core computes QKV projections for its head slice (full x),
attention with the lag bias for its 2 heads over both batch elements,
and a partial output projection (its 128 rows of wo). Host sums the 8
partials and adds bo (+ bv @ wo, folded off-device: softmax rows sum to
1, so V's bias contributes a constant row).

v2 layout (all matmuls bf16, PSUM fp32):
  xT      (1024, 4096)  x transposed, tok = b*2048 + s
  QT/KT   (128, 4096)   q^T/k^T, partitions = [h0 dk(64) | h1 dk(64)]
  Vb      (128, 32, 130) per 128-tok chunk: [V_h0(64) | 1 | V_h1(64) | 1]
  scores  transposed S^T (k on partitions, q free); the two heads run as
          one concurrent row-group pair on the PE (rows 0-63 / 64-127).
  bias    host-precomputed EB_h = exp(bias) (2048,2048) bf16; applied as
          exp(s)*EB on DVE in bf16 2x mode (removes the fp32 PSUM add
          from the critical chain that kept the PE clock-throttled).
  denom   via ones-columns of Vb; reciprocal_approx_fast on DVE, then a
          1-row f32r broadcast matmul + one TT multiply per (qc, b)
          normalizes OTp -> OTn, overlapping the next q-chunk.
  outproj OTn packed (h0 rows 0-63, h1 rows 64-127) -> single
          128-contraction matmul pair per 128-token chunk; PSUM drained
          straight to DRAM by DMA (fp32).
"""

import ml_dtypes
import numpy as np
from contextlib import ExitStack

import concourse.bass as bass
import concourse.bacc as bacc
import concourse.mybir as mybir
import concourse.tile as tile
from concourse.bass_utils import run_bass_kernel_spmd
from concourse.masks import make_identity

F32 = mybir.dt.float32
F32R = mybir.dt.float32r
BF16 = mybir.dt.bfloat16
AF = mybir.ActivationFunctionType

N_CORES = 8
B, S, D = 2, 2048, 1024
H, DK = 16, 64
TOK = B * S              # 4096
NQ = 512                 # q-chunk (matmul free dim)
NQC = S // NQ            # 4 q-chunks per batch
NJ = S // 128            # 16 k-chunks per batch
DCH = D // 128           # 8 contraction chunks

# Set by test.py for profiling; harness leaves these untouched.
TRACE = False
TRACE_DIR = None

_CACHED_NC = None

# optional debug hook: callback(name, tile_ap) — used by kernel_dbg.py
_DBG = None


def _dbg(name, t):
    if _DBG is not None:
        _DBG(name, t)


def _body(ctx: ExitStack, tc, aps):
    nc = tc.nc
    xT, wq, wk, wv, bq, bk, wo, EB0, EB1, out = (
        aps["xT"], aps["wq"], aps["wk"], aps["wv"], aps["bq"], aps["bk"],
        aps["wo"], aps["EB0"], aps["EB1"], aps["out"])
    EBh = [EB0, EB1]

    const = ctx.enter_context(tc.tile_pool(name="const", bufs=1))
    persist = ctx.enter_context(tc.tile_pool(name="persist", bufs=1))
    spool = ctx.enter_context(tc.tile_pool(name="spsum", bufs=2, space="PSUM"))
    opool = ctx.enter_context(tc.tile_pool(name="opsum", bufs=4, space="PSUM"))

    # ---- constants ----
    identf = const.tile([128, 128], F32, tag="idf")
    make_identity(nc, identf[:])
    ident = const.tile([128, 128], BF16, tag="id")
    nc.scalar.copy(ident[:], identf[:])
    w_sb = {}
    for name, ap in (("q", wq), ("k", wk), ("v", wv)):
        t = const.tile([128, DCH, 128], BF16, tag=f"w{name}")
        nc.sync.dma_start(t[:], ap.rearrange("(c p) m -> p c m", p=128))
        w_sb[name] = t
    b_sb = {}
    for name, ap in (("q", bq), ("k", bk)):
        t = const.tile([128, 1], F32, tag=f"b{name}")
        nc.sync.dma_start(t[:], ap[:])
        b_sb[name] = t
    wo_sb = const.tile([128, D], BF16, tag="wo")
    nc.sync.dma_start(wo_sb[:], wo[:])
    # ones rows at partitions 0 and 64 for the per-head denominator
    # broadcast matmul (bf16; memset f32 then cast once)
    ones65f = const.tile([65, 64], F32, tag="ones65f")
    nc.vector.memset(ones65f[:], 1.0)
    ones65 = const.tile([65, 64], BF16, tag="ones65")
    nc.scalar.copy(ones65[:], ones65f[:])
    # preload the exp table set before the attention phase needs it
    warm = const.tile([1, 8], F32, tag="warm")
    nc.vector.memset(warm[:], 0.0)
    nc.scalar.activation(warm[:], warm[:], AF.Exp)

    # ---- persistent activations ----
    QT = persist.tile([128, TOK], BF16, tag="QT")
    KT = persist.tile([128, TOK], BF16, tag="KT")
    Vb = persist.tile([128, TOK // 128, 130], BF16, tag="Vb")
    OTp = persist.tile([128, TOK], BF16, tag="OTp")
    OTn = persist.tile([128, TOK], BF16, tag="OTn")
    # Z rows live at partitions 0 (h0) and 64 (h1); the custom-DVE
    # reciprocal runs SBUF->SBUF over the whole 65-row block (HW silently
    # corrupts PSUM-source / base-32 custom ops). Rows 1-63 are padding:
    # memset 1 so recip(1)=1 is defined.
    zsb = persist.tile([65, TOK], F32, tag="zsb")
    rec = persist.tile([65, TOK], F32, tag="rec")
    recb = persist.tile([65, TOK], BF16, tag="recb")
    nc.gpsimd.memset(zsb[:], 1.0)
    for _n, _t in (("QT", QT), ("KT", KT), ("Vb", Vb), ("OTp", OTp),
                   ("OTn", OTn), ("recb", recb), ("zsb", zsb)):
        _dbg(_n, _t)

    # ones columns of Vb (positions 64 and 129 of each 130-stripe)
    nc.vector.memset(Vb[:, :, 64:65], 1.0)
    nc.vector.memset(Vb[:, :, 129:130], 1.0)

    # ---- phases 1-2: QKV projections + V transpose (scoped pools) ----
    with tc.tile_pool(name="xin", bufs=3) as xpool, \
         tc.tile_pool(name="vtp", bufs=1) as vtpool:
        VT = vtpool.tile([128, TOK], BF16, tag="VT")
        xT_r = xT.rearrange("(c p) n -> p c n", p=128)
        for t in range(TOK // NQ):
            xt = xpool.tile([128, DCH, NQ], BF16, tag="x")
            nc.sync.dma_start(xt[:], xT_r[:, :, t * NQ:(t + 1) * NQ])
            sl = slice(t * NQ, (t + 1) * NQ)
            for name, dst in (("q", QT), ("k", KT), ("v", VT)):
                ps = opool.tile([128, NQ], F32, tag="o", name="ps_proj")
                for d in range(DCH):
                    nc.tensor.matmul(ps[:], w_sb[name][:, d, :], xt[:, d, :],
                                     start=(d == 0), stop=(d == DCH - 1))
                if name == "v":
                    nc.vector.tensor_copy(dst[:, sl], ps[:])
                else:
                    nc.scalar.activation(dst[:, sl], ps[:], AF.Identity,
                                         bias=b_sb[name][:])

        # V transpose into (tok, hd) chunks
        for u in range(TOK // 128):
            pt = opool.tile([128, 128], BF16, tag="o", name="pt_tr")
            nc.tensor.transpose(pt[:], VT[:, u * 128:(u + 1) * 128], ident[:])
            dst = Vb[:, u, :].rearrange("p (g x) -> p g x", g=2)[:, :, 0:64]
            src = pt[:].rearrange("p (g x) -> p g x", g=2)
            if u % 2 == 0:
                nc.scalar.copy(dst, src)
            else:
                nc.vector.tensor_copy(dst, src)

    # ---- phase 3: attention ----
    bpool = ctx.enter_context(tc.tile_pool(name="bin", bufs=6))
    ppool = ctx.enter_context(tc.tile_pool(name="pexp", bufs=6))
    EB_r = [EBh[h].rearrange("(j p) q -> p j q", p=128) for h in range(2)]

    def make_finale(q0):
        # PE + final-TT part of the normalize epilogue. Deferred into the
        # middle of the NEXT block: the PE is in-order, so emitting the R
        # matmuls right after the block would stall it on the DVE
        # reciprocal chain.
        def finale():
            R_ps = opool.tile([128, NQ], F32, tag="o", name="R_ps")
            for hh in range(2):
                nc.tensor.matmul(
                    R_ps[64 * hh:64 * hh + 64, :],
                    ones65[64 * hh:64 * hh + 1, :],
                    recb[64 * hh:64 * hh + 1, q0:q0 + NQ],
                    start=True, stop=True)
            nc.vector.tensor_mul(OTn[:, q0:q0 + NQ], OTp[:, q0:q0 + NQ],
                                 R_ps[:])
        return finale

    pend = []
    for qc in range(NQC):
        # bias stripe for this q-chunk in 4 quarter tiles (1 MB DMA
        # granularity keeps the prefetch pipeline fine-grained); reused by
        # both batch halves (batch is the outer loop so each half's
        # normalize epilogue overlaps the other half's compute)
        ebq = []
        for jq in range(4):
            t = bpool.tile([128, 4, 2, NQ], BF16, tag="b", name="ebstr")
            for hh in range(2):
                nc.sync.dma_start(
                    t[:, :, hh, :],
                    EB_r[hh][:, jq * 4:(jq + 1) * 4, qc * NQ:(qc + 1) * NQ])
            ebq.append(t)
        for b in range(2):
            q0 = b * S + qc * NQ
            O_ps = [opool.tile([65, NQ], F32, tag="o", name=f"O_ps{hh}")
                    for hh in range(2)]
            for j in range(NJ):
                k0 = b * S + j * 128
                # head-packed scores: h0 rows 0-63, h1 rows 64-127 run
                # concurrently on disjoint PE row groups
                sps = spool.tile([128, 2 * NQ], F32, tag="s")
                for hh in range(2):
                    nc.tensor.matmul(
                        sps[:, hh * NQ:(hh + 1) * NQ],
                        KT[64 * hh:64 * hh + 64, k0:k0 + 128],
                        QT[64 * hh:64 * hh + 64, q0:q0 + NQ],
                        start=True, stop=True)
                pe = ppool.tile([128, 2 * NQ], BF16, tag="p")
                nc.scalar.activation(pe[:], sps[:], AF.Exp)
                # lag bias as exp(bias) multiply, bf16 2x DVE mode
                nc.vector.tensor_mul(
                    pe[:], pe[:],
                    ebq[j // 4][:, j % 4, :, :].rearrange("p g q -> p (g q)"))
                if (qc, b, j) == (0, 0, 0):
                    _dbg("pe0", pe)
                if j == 2 and pend:
                    pend.pop(0)()
                for hh in range(2):
                    nc.tensor.matmul(
                        O_ps[hh][:],
                        Vb[:, b * NJ + j, 65 * hh:65 * hh + 65],
                        pe[:, hh * NQ:(hh + 1) * NQ],
                        start=(j == 0), stop=(j == NJ - 1))
            # immediate epilogue: stash unnormalized O^T + Z rows, then the
            # batched reciprocal — all on DVE; ScalarE is saturated by exp
            nc.vector.tensor_copy(OTp[0:64, q0:q0 + NQ], O_ps[0][0:64, :])
            nc.vector.tensor_copy(OTp[64:128, q0:q0 + NQ], O_ps[1][0:64, :])
            nc.vector.tensor_copy(zsb[0:1, q0:q0 + NQ], O_ps[0][64:65, :])
            nc.vector.tensor_copy(zsb[64:65, q0:q0 + NQ], O_ps[1][64:65, :])
            nc.vector.reciprocal_approx_fast(rec[:, q0:q0 + NQ],
                                             zsb[:, q0:q0 + NQ])
            nc.vector.tensor_copy(recb[:, q0:q0 + NQ], rec[:, q0:q0 + NQ])
            pend.append(make_finale(q0))
    for f in pend:
        f()

    # ---- phase 4: output projection (token chunks ordered by the qc that
    # produced them, so the first matmuls never wait on the last finale) ----
    dpool = ctx.enter_context(tc.tile_pool(name="drain", bufs=4))
    u_order = [u for qc in range(NQC) for u in
               (list(range(4 * qc, 4 * qc + 4))
                + list(range(16 + 4 * qc, 16 + 4 * qc + 4)))]
    for u in u_order:
        ps = spool.tile([128, 2 * NQ], F32, tag="s")
        for half in range(2):
            osl = slice(half * NQ, (half + 1) * NQ)
            nc.tensor.matmul(ps[:, osl], OTn[:, u * 128:(u + 1) * 128],
                             wo_sb[:, osl], start=True, stop=True)
        osb = dpool.tile([128, 2 * NQ], BF16, tag="osb")
        # alternate engines so the drain isn't serialized on one of them
        if u % 2 == 0:
            nc.scalar.copy(osb[:], ps[:])
        else:
            nc.vector.tensor_copy(osb[:], ps[:])
        nc.sync.dma_start(out[u * 128:(u + 1) * 128, :], osb[:])


def build_program():
    nc = bacc.Bacc("TRN2", target_bir_lowering=False, debug=False,
                   enable_asserts=False, num_devices=N_CORES)
    aps = {}
    specs = [
        ("xT", (D, TOK), BF16), ("wq", (D, 128), BF16), ("wk", (D, 128), BF16),
        ("wv", (D, 128), BF16), ("bq", (128, 1), F32), ("bk", (128, 1), F32),
        ("wo", (128, D), BF16), ("EB0", (S, S), BF16), ("EB1", (S, S), BF16),
    ]
    for name, shape, dt in specs:
        aps[name] = nc.dram_tensor(name, shape, dt, kind="ExternalInput").ap()
    aps["out"] = nc.dram_tensor("out", (TOK, D), BF16,
                                kind="ExternalOutput").ap()
    with tile.TileContext(nc) as tc:
        with ExitStack() as ctx:
            _body(ctx, tc, aps)
    nc.compile()
    return nc


def _get_nc():
    global _CACHED_NC
    if _CACHED_NC is None:
        _CACHED_NC = build_program()
    return _CACHED_NC


def _host_prep(x, lag, wq, bq, wk, bk, wv, bv, wo, bo, lag_bias):
    x = np.asarray(x, dtype=np.float32)
    lag = np.asarray(lag).astype(np.int64)
    xT = np.ascontiguousarray(x.reshape(TOK, D).T.astype(ml_dtypes.bfloat16))
    ld = np.abs(lag[:, None] - lag[None, :]).astype(np.int64)
    lag_bias = np.asarray(lag_bias, dtype=np.float32)
    scale = np.float32(1.0 / np.sqrt(DK))
    wq = np.asarray(wq, dtype=np.float32) * scale
    bq = np.asarray(bq, dtype=np.float32) * scale
    in_maps = []
    for c in range(N_CORES):
        sl = slice(c * 128, (c + 1) * 128)
        in_maps.append({
            "xT": xT,
            "wq": np.ascontiguousarray(wq[:, sl].astype(ml_dtypes.bfloat16)),
            "wk": np.ascontiguousarray(
                np.asarray(wk, np.float32)[:, sl].astype(ml_dtypes.bfloat16)),
            "wv": np.ascontiguousarray(
                np.asarray(wv, np.float32)[:, sl].astype(ml_dtypes.bfloat16)),
            "bq": np.ascontiguousarray(bq[sl].reshape(128, 1)),
            "bk": np.ascontiguousarray(
                np.asarray(bk, np.float32)[sl].reshape(128, 1)),
            "wo": np.ascontiguousarray(
                np.asarray(wo, np.float32)[sl, :].astype(ml_dtypes.bfloat16)),
            "EB0": np.ascontiguousarray(
                np.exp(lag_bias[2 * c][ld]).astype(ml_dtypes.bfloat16)),
            "EB1": np.ascontiguousarray(
                np.exp(lag_bias[2 * c + 1][ld]).astype(ml_dtypes.bfloat16)),
        })
    return in_maps


def kernel(x, lag, wq, bq, wk, bk, wv, bv, wo, bo, lag_bias):
    nc = _get_nc()
    in_maps = _host_prep(x, lag, wq, bq, wk, bk, wv, bv, wo, bo, lag_bias)
    kwargs = {}
    if TRACE:
        kwargs = dict(trace=True, tmpdir=TRACE_DIR)
    res = run_bass_kernel_spmd(nc, in_maps, core_ids=list(range(N_CORES)),
                               **kwargs)
    if TRACE:
        print(f"HW exec time: {res.exec_time_ns} ns")
    total = res.results[0]["out"].astype(np.float32)
    for c in range(1, N_CORES):
        total += res.results[c]["out"]
    # bo plus the folded V-bias contribution (softmax rows sum to 1)
    total += (np.asarray(bo, np.float32)
              + np.asarray(bv, np.float32) @ np.asarray(wo, np.float32))[None, :]
    return total.reshape(B, S, D)


# revision 25
# speedup vs baseline: 1.1803x; 1.1803x over previous
"""MultiHeadSelfAttentionWithLagBias on 8 TRN2 NeuronCores.

Sharding: tensor-parallel over heads — 16 heads / 8 cores = 2 heads per
core. Each core computes QKV projections for its head slice (full x),
attention with the lag bias for its 2 heads over both batch elements,
and a partial output projection (its 128 rows of wo). Host sums the 8
partials and adds bo (+ bv @ wo, folded off-device: softmax rows sum to
1, so V's bias contributes a constant row).

v2 layout (all matmuls bf16, PSUM fp32):
  xT      (1024, 4096)  x transposed, tok = b*2048 + s
  QT/KT   (128, 4096)   q^T/k^T, partitions = [h0 dk(64) | h1 dk(64)]
  Vb      (128, 32, 130) per 128-tok chunk: [V_h0(64) | 1 | V_h1(64) | 1]
  scores  transposed S^T (k on partitions, q free); the two heads run as
          one concurrent row-group pair on the PE (rows 0-63 / 64-127).
  bias    host-precomputed EB_h = exp(bias) (2048,2048) bf16; applied as
          exp(s)*EB on DVE in bf16 2x mode (removes the fp32 PSUM add
          from the critical chain that kept the PE clock-throttled).
  denom   via ones-columns of Vb; reciprocal_approx_fast on DVE, then a
          1-row f32r broadcast matmul + one TT multiply per (qc, b)
          normalizes OTp -> OTn, overlapping the next q-chunk.
  outproj OTn packed (h0 rows 0-63, h1 rows 64-127) -> single
          128-contraction matmul pair per 128-token chunk; PSUM drained
          straight to DRAM by DMA (fp32).
"""

import ml_dtypes
import numpy as np
from contextlib import ExitStack

import concourse.bass as bass
import concourse.bacc as bacc
import concourse.mybir as mybir
import concourse.tile as tile
from concourse.bass_utils import run_bass_kernel_spmd
from concourse.masks import make_identity

F32 = mybir.dt.float32
F32R = mybir.dt.float32r
BF16 = mybir.dt.bfloat16
AF = mybir.ActivationFunctionType

N_CORES = 8
B, S, D = 2, 2048, 1024
H, DK = 16, 64
TOK = B * S              # 4096
NQ = 512                 # q-chunk (matmul free dim)
NQC = S // NQ            # 4 q-chunks per batch
NJ = S // 128            # 16 k-chunks per batch
DCH = D // 128           # 8 contraction chunks

# Set by test.py for profiling; harness leaves these untouched.
TRACE = False
TRACE_DIR = None

_CACHED_NC = None

# optional debug hook: callback(name, tile_ap) — used by kernel_dbg.py
_DBG = None


def _dbg(name, t):
    if _DBG is not None:
        _DBG(name, t)


def _body(ctx: ExitStack, tc, aps):
    nc = tc.nc
    xT, wq, wk, wv, bq, bk, wo, EB0, EB1, out = (
        aps["xT"], aps["wq"], aps["wk"], aps["wv"], aps["bq"], aps["bk"],
        aps["wo"], aps["EB0"], aps["EB1"], aps["out"])
    EBh = [EB0, EB1]

    const = ctx.enter_context(tc.tile_pool(name="const", bufs=1))
    persist = ctx.enter_context(tc.tile_pool(name="persist", bufs=1))
    spool = ctx.enter_context(tc.tile_pool(name="spsum", bufs=2, space="PSUM"))
    opool = ctx.enter_context(tc.tile_pool(name="opsum", bufs=2, space="PSUM"))

    # ---- constants ----
    identf = const.tile([128, 128], F32, tag="idf")
    make_identity(nc, identf[:])
    ident = const.tile([128, 128], BF16, tag="id")
    nc.scalar.copy(ident[:], identf[:])
    w_sb = {}
    for name, ap in (("q", wq), ("k", wk), ("v", wv)):
        t = const.tile([128, DCH, 128], BF16, tag=f"w{name}")
        nc.sync.dma_start(t[:], ap.rearrange("(c p) m -> p c m", p=128))
        w_sb[name] = t
    b_sb = {}
    for name, ap in (("q", bq), ("k", bk)):
        t = const.tile([128, 1], F32, tag=f"b{name}")
        nc.sync.dma_start(t[:], ap[:])
        b_sb[name] = t
    wo_sb = const.tile([128, D], BF16, tag="wo")
    nc.sync.dma_start(wo_sb[:], wo[:])
    # ones rows at partitions 0 and 64 for the per-head denominator
    # broadcast matmul (bf16; memset f32 then cast once)
    ones65f = const.tile([65, 64], F32, tag="ones65f")
    nc.vector.memset(ones65f[:], 1.0)
    ones65 = const.tile([65, 64], BF16, tag="ones65")
    nc.scalar.copy(ones65[:], ones65f[:])
    # preload the exp table set before the attention phase needs it
    warm = const.tile([1, 8], F32, tag="warm")
    nc.vector.memset(warm[:], 0.0)
    nc.scalar.activation(warm[:], warm[:], AF.Exp)

    # ---- persistent activations ----
    QT = persist.tile([128, TOK], BF16, tag="QT")
    KT = persist.tile([128, TOK], BF16, tag="KT")
    Vb = persist.tile([128, TOK // 128, 130], BF16, tag="Vb")
    OTp = persist.tile([128, TOK], BF16, tag="OTp")
    OTn = persist.tile([128, TOK], BF16, tag="OTn")
    # Z rows live at partitions 0 (h0) and 64 (h1); the custom-DVE
    # reciprocal runs SBUF->SBUF over the whole 65-row block (HW silently
    # corrupts PSUM-source / base-32 custom ops). Rows 1-63 are padding:
    # memset 1 so recip(1)=1 is defined.
    zsb = persist.tile([65, TOK], F32, tag="zsb")
    rec = persist.tile([65, TOK], F32, tag="rec")
    recb = persist.tile([65, TOK], BF16, tag="recb")
    nc.gpsimd.memset(zsb[:], 1.0)
    for _n, _t in (("QT", QT), ("KT", KT), ("Vb", Vb), ("OTp", OTp),
                   ("OTn", OTn), ("recb", recb), ("zsb", zsb)):
        _dbg(_n, _t)

    # ones columns of Vb (positions 64 and 129 of each 130-stripe)
    nc.vector.memset(Vb[:, :, 64:65], 1.0)
    nc.vector.memset(Vb[:, :, 129:130], 1.0)

    # ---- macro-pipelined phases ----
    # b=0 QKV runs up front; b=0 attention overlaps b=1 QKV/transposes
    # (injected into the PE stream mid-loop); b=1 attention overlaps the
    # output projection of b=0's tokens; a short tail drains the rest.
    bpool = ctx.enter_context(tc.tile_pool(name="bin", bufs=5))
    ppool = ctx.enter_context(tc.tile_pool(name="pexp", bufs=6))
    xpool = ctx.enter_context(tc.tile_pool(name="xin", bufs=2))
    dpool = ctx.enter_context(tc.tile_pool(name="drain", bufs=4))
    jpool = ctx.enter_context(tc.tile_pool(name="jps", bufs=1, space="PSUM"))
    EB_r = [EBh[h].rearrange("(j p) q -> p j q", p=128) for h in range(2)]
    xT_r = xT.rearrange("(c p) n -> p c n", p=128)
    VT = persist.tile([128, TOK], BF16, tag="VT")

    def emit_xdma(t):
        xt = xpool.tile([128, DCH, NQ], BF16, tag="x", name="xt")
        nc.sync.dma_start(xt[:], xT_r[:, :, t * NQ:(t + 1) * NQ])
        return xt

    def emit_q_half(t, xt, pool, tag):
        sl = slice(t * NQ, (t + 1) * NQ)
        qk = pool.tile([128, 2 * NQ], F32, tag=tag, name="qk_ps")
        for d in range(DCH):
            nc.tensor.matmul(qk[:, 0:NQ], w_sb["q"][:, d, :], xt[:, d, :],
                             start=(d == 0), stop=(d == DCH - 1))
        nc.scalar.activation(QT[:, sl], qk[:, 0:NQ], AF.Identity,
                             bias=b_sb["q"][:])
        return qk

    def emit_k_half(t, xt, qk):
        sl = slice(t * NQ, (t + 1) * NQ)
        for d in range(DCH):
            nc.tensor.matmul(qk[:, NQ:2 * NQ], w_sb["k"][:, d, :], xt[:, d, :],
                             start=(d == 0), stop=(d == DCH - 1))
        nc.scalar.activation(KT[:, sl], qk[:, NQ:2 * NQ], AF.Identity,
                             bias=b_sb["k"][:])

    def emit_v(t, xt, pool, tag):
        sl = slice(t * NQ, (t + 1) * NQ)
        vps = pool.tile([128, NQ], F32, tag=tag, name="v_ps")
        for d in range(DCH):
            nc.tensor.matmul(vps[:], w_sb["v"][:, d, :], xt[:, d, :],
                             start=(d == 0), stop=(d == DCH - 1))
        nc.vector.tensor_copy(VT[:, sl], vps[:])

    def emit_pt(u, pool, tag):
        pt = pool.tile([128, 128], BF16, tag=tag, name="pt_tr")
        nc.tensor.transpose(pt[:], VT[:, u * 128:(u + 1) * 128], ident[:])
        dst = Vb[:, u, :].rearrange("p (g x) -> p g x", g=2)[:, :, 0:64]
        src = pt[:].rearrange("p (g x) -> p g x", g=2)
        if u % 2 == 0:
            nc.scalar.copy(dst, src)
        else:
            nc.vector.tensor_copy(dst, src)

    ucount = [0]
    emitted_u = set()

    def emit_uchunk(u, pool, tag):
        emitted_u.add(u)
        ps = pool.tile([128, 2 * NQ], F32, tag=tag, name="u_ps")
        for half in range(2):
            osl = slice(half * NQ, (half + 1) * NQ)
            nc.tensor.matmul(ps[:, osl], OTn[:, u * 128:(u + 1) * 128],
                             wo_sb[:, osl], start=True, stop=True)
        osb = dpool.tile([128, 2 * NQ], BF16, tag="osb")
        if ucount[0] % 2 == 0:
            nc.scalar.copy(osb[:], ps[:])
        else:
            nc.vector.tensor_copy(osb[:], ps[:])
        ucount[0] += 1
        nc.sync.dma_start(out[u * 128:(u + 1) * 128, :], osb[:])

    # -- initial phase: QKV + transposes for batch 0 --
    for t in range(4):
        xt = emit_xdma(t)
        qk = emit_q_half(t, xt, spool, "s")
        emit_k_half(t, xt, qk)
        emit_v(t, xt, opool, "o")
        for i in range(4):
            emit_pt(4 * t + i, opool, "o")

    def attention_block(b, qc, inject, absorber):
        q0 = b * S + qc * NQ
        ebq = []
        for jq in range(4):
            bt = bpool.tile([128, 4, 2, NQ], BF16, tag="b", name="ebstr")
            for hh in range(2):
                nc.sync.dma_start(
                    bt[:, :, hh, :],
                    EB_r[hh][:, jq * 4:(jq + 1) * 4, qc * NQ:(qc + 1) * NQ])
            ebq.append(bt)
        O_ps = [opool.tile([65, NQ], F32, tag="o", name=f"O_ps{hh}")
                for hh in range(2)]
        for j in range(NJ):
            k0 = b * S + j * 128
            sps = spool.tile([128, 2 * NQ], F32, tag="s")
            for hh in range(2):
                nc.tensor.matmul(
                    sps[:, hh * NQ:(hh + 1) * NQ],
                    KT[64 * hh:64 * hh + 64, k0:k0 + 128],
                    QT[64 * hh:64 * hh + 64, q0:q0 + NQ],
                    start=True, stop=True)
            pe = ppool.tile([128, 2 * NQ], BF16, tag="p")
            nc.scalar.activation(pe[:], sps[:], AF.Exp)
            nc.vector.tensor_mul(
                pe[:], pe[:],
                ebq[j // 4][:, j % 4, :, :].rearrange("p g q -> p (g q)"))
            if (b, qc, j) == (0, 0, 0):
                _dbg("pe0", pe)
            if j in inject:
                inject[j]()
            for hh in range(2):
                nc.tensor.matmul(
                    O_ps[hh][:],
                    Vb[:, b * NJ + j, 65 * hh:65 * hh + 65],
                    pe[:, hh * NQ:(hh + 1) * NQ],
                    start=(j == 0), stop=(j == NJ - 1))
        # epilogue: stash O^T + Z rows (ACT/DVE split keeps the serial
        # chain short), batched reciprocal + bf16 cast
        nc.vector.tensor_copy(OTp[0:64, q0:q0 + NQ], O_ps[0][0:64, :])
        nc.scalar.copy(OTp[64:128, q0:q0 + NQ], O_ps[1][0:64, :])
        nc.scalar.copy(zsb[0:1, q0:q0 + NQ], O_ps[0][64:65, :])
        nc.vector.tensor_copy(zsb[64:65, q0:q0 + NQ], O_ps[1][64:65, :])
        nc.vector.reciprocal_approx_fast(rec[:, q0:q0 + NQ],
                                         zsb[:, q0:q0 + NQ])
        nc.vector.tensor_copy(recb[:, q0:q0 + NQ], rec[:, q0:q0 + NQ])
        # absorber: injected PE work that hides the reciprocal chain
        # latency before the R matmuls need it
        for f in absorber:
            f()
        R_ps = opool.tile([128, NQ], F32, tag="o", name="R_ps")
        for hh in range(2):
            nc.tensor.matmul(
                R_ps[64 * hh:64 * hh + 64, :],
                ones65[64 * hh:64 * hh + 1, :],
                recb[64 * hh:64 * hh + 1, q0:q0 + NQ],
                start=True, stop=True)
        nc.vector.tensor_mul(OTn[:, q0:q0 + NQ], OTp[:, q0:q0 + NQ],
                             R_ps[:])

    # -- batch 0 attention, batch 1 QKV injected --
    for qc in range(NQC):
        t = 4 + qc
        xt = emit_xdma(t)
        st = {"qk": None}

        def i_q(t=t, xt=xt, st=st):
            st["qk"] = emit_q_half(t, xt, jpool, "j")

        def i_k(t=t, xt=xt, st=st):
            emit_k_half(t, xt, st["qk"])

        def i_v(t=t, xt=xt):
            emit_v(t, xt, jpool, "j")

        absorber = [lambda u=16 + 4 * qc + i: emit_pt(u, jpool, "j")
                    for i in range(4)]
        attention_block(0, qc, {4: i_q, 7: i_k, 10: i_v}, absorber)

    # -- batch 1 attention, batch 0 output projection injected --
    for qc in range(NQC):
        us = list(range(4 * qc, 4 * qc + 4))          # batch-0 chunks
        if qc >= 1:
            extra = [16 + 4 * (qc - 1), 17 + 4 * (qc - 1)]  # early b1 chunks
        else:
            extra = []
        inject = {3: (lambda u=us[0]: emit_uchunk(u, jpool, "j")),
                  7: (lambda u=us[1]: emit_uchunk(u, jpool, "j")),
                  11: (lambda u=us[2]: emit_uchunk(u, jpool, "j"))}
        absorber = [lambda u=us[3]: emit_uchunk(u, jpool, "j")]
        absorber += [lambda u=u: emit_uchunk(u, jpool, "j") for u in extra]
        attention_block(1, qc, inject, absorber)

    # -- tail: remaining batch-1 projection chunks --
    rest = [u for u in range(16, 32) if u not in emitted_u]
    for i, u in enumerate(rest):
        emit_uchunk(u, spool if i % 2 == 0 else jpool,
                    "s" if i % 2 == 0 else "j")


def build_program():
    nc = bacc.Bacc("TRN2", target_bir_lowering=False, debug=False,
                   enable_asserts=False, num_devices=N_CORES)
    aps = {}
    specs = [
        ("xT", (D, TOK), BF16), ("wq", (D, 128), BF16), ("wk", (D, 128), BF16),
        ("wv", (D, 128), BF16), ("bq", (128, 1), F32), ("bk", (128, 1), F32),
        ("wo", (128, D), BF16), ("EB0", (S, S), BF16), ("EB1", (S, S), BF16),
    ]
    for name, shape, dt in specs:
        aps[name] = nc.dram_tensor(name, shape, dt, kind="ExternalInput").ap()
    aps["out"] = nc.dram_tensor("out", (TOK, D), BF16,
                                kind="ExternalOutput").ap()
    with tile.TileContext(nc) as tc:
        with ExitStack() as ctx:
            _body(ctx, tc, aps)
    nc.compile()
    return nc


def _get_nc():
    global _CACHED_NC
    if _CACHED_NC is None:
        _CACHED_NC = build_program()
    return _CACHED_NC


def _host_prep(x, lag, wq, bq, wk, bk, wv, bv, wo, bo, lag_bias):
    x = np.asarray(x, dtype=np.float32)
    lag = np.asarray(lag).astype(np.int64)
    xT = np.ascontiguousarray(x.reshape(TOK, D).T.astype(ml_dtypes.bfloat16))
    ld = np.abs(lag[:, None] - lag[None, :]).astype(np.int64)
    lag_bias = np.asarray(lag_bias, dtype=np.float32)
    scale = np.float32(1.0 / np.sqrt(DK))
    wq = np.asarray(wq, dtype=np.float32) * scale
    bq = np.asarray(bq, dtype=np.float32) * scale
    in_maps = []
    for c in range(N_CORES):
        sl = slice(c * 128, (c + 1) * 128)
        in_maps.append({
            "xT": xT,
            "wq": np.ascontiguousarray(wq[:, sl].astype(ml_dtypes.bfloat16)),
            "wk": np.ascontiguousarray(
                np.asarray(wk, np.float32)[:, sl].astype(ml_dtypes.bfloat16)),
            "wv": np.ascontiguousarray(
                np.asarray(wv, np.float32)[:, sl].astype(ml_dtypes.bfloat16)),
            "bq": np.ascontiguousarray(bq[sl].reshape(128, 1)),
            "bk": np.ascontiguousarray(
                np.asarray(bk, np.float32)[sl].reshape(128, 1)),
            "wo": np.ascontiguousarray(
                np.asarray(wo, np.float32)[sl, :].astype(ml_dtypes.bfloat16)),
            "EB0": np.ascontiguousarray(
                np.exp(lag_bias[2 * c][ld]).astype(ml_dtypes.bfloat16)),
            "EB1": np.ascontiguousarray(
                np.exp(lag_bias[2 * c + 1][ld]).astype(ml_dtypes.bfloat16)),
        })
    return in_maps


def kernel(x, lag, wq, bq, wk, bk, wv, bv, wo, bo, lag_bias):
    nc = _get_nc()
    in_maps = _host_prep(x, lag, wq, bq, wk, bk, wv, bv, wo, bo, lag_bias)
    kwargs = {}
    if TRACE:
        kwargs = dict(trace=True, tmpdir=TRACE_DIR)
    res = run_bass_kernel_spmd(nc, in_maps, core_ids=list(range(N_CORES)),
                               **kwargs)
    if TRACE:
        print(f"HW exec time: {res.exec_time_ns} ns")
    total = res.results[0]["out"].astype(np.float32)
    for c in range(1, N_CORES):
        total += res.results[c]["out"]
    # bo plus the folded V-bias contribution (softmax rows sum to 1)
    total += (np.asarray(bo, np.float32)
              + np.asarray(bv, np.float32) @ np.asarray(wo, np.float32))[None, :]
    return total.reshape(B, S, D)
